# revision 62
# baseline (speedup 1.0000x reference)
"""Blockwise reconditioner (block-16 normalization) on 8 Trainium2 cores.

Math per row r, block g (block size 16):
    mean = mean(x[r, 16g:16g+16])
    var  = sum((x - mean)^2) / 15          (unbiased, ddof=1)
    out  = (x - mean) / sqrt(var + 1e-5) * scales[g] + shifts[g]

Implemented as out = x * a + b with per-block coefficients
    a = scales[g] / sqrt(var + eps)
    b = shifts[g] - mean * a
using raw = sum(x^2) - s1^2/16, var = raw/15 (s1 = block sum).

bf16 data path (tolerance is 2e-2; bf16 end-to-end measures ~3.4e-3):
host casts x fp32 -> bf16, device computes in bf16/fp32, host casts the
bf16 output back to fp32.  Halves HBM traffic (DMA roofline ~94us fp32
-> ~47us bf16) and enables the DVE 2x perf mode for the apply.

Per-core pipeline over 8 flattened [128, 4096] chunks, software
pipelined with a 1-chunk lag between a chunk's "head" (DMA in, s1
stats, PE transposes, ACT squares) and its "tail" (PE masked matmuls,
coefficient math, apply, DMA out).  The lag keeps the head-stage and
tail-stage SBUF streams time-shifted — concurrent same-tile streams
contend for SBUF bandwidth and stretch every engine's ops ~1.5x — and
keeps each engine's in-order queue free of cross-engine stalls.

s1 (block sums) alternate per chunk between two engines to balance
load: GpSimd (4-pass pairwise-halves TENSOR_TENSOR adder tree;
InstPool/STT are invalid on the Pool engine, and TensorReduce gets no
bf16 speedup on the DVE) and ACT (copy the PE-transposed raw data
PSUM->SBUF; the Tensor engine then mask-reduces it — it issues a
matmul every ~55ns fully pipelined, so extra matmul passes are nearly
free).  s2 always: ACT squares PSUM->SBUF bf16, then 32 matmuls
(stationary=sqT_k, moving=mask_k) accumulate row-major [row, block]
sums directly in PSUM (no flip back needed).

Coefficients per chunk: raw = s2 - s1^2/16 (STT against PSUM),
sd = sqrt(raw/15 + eps) (ACT), rstd via the custom-DVE fast
reciprocal, a/b written DUPLICATED (a[2g]=a[2g+1]) in bf16 so the
apply's broadcast AP keeps a packed 2-byte last dim; b is fused via
the AFFINE_MUL_REDUCE custom op + one packed-bf16 add.  The apply
out = x*a + b runs in place as two DVE TENSOR_TENSOR passes in 2x_1p
mode, then DMA out.

Measured: 161.3us fp32 baseline -> 109.6us (DVE is the end-to-end
governor: ~26us to first coefficient, then ~78us dense DVE work).
"""

import sys

import numpy as np
import ml_dtypes

for _p in ("/opt/trn_rl_repo",):
    if _p not in sys.path:
        sys.path.insert(0, _p)

import concourse.bacc as bacc
import concourse.bass as bass
import concourse.tile as tile
from concourse import mybir
from concourse.bass_utils import run_bass_kernel_spmd

F32 = mybir.dt.float32
BF16 = mybir.dt.bfloat16
ALU = mybir.AluOpType

N_CORES = 8
B_FULL = 4096          # total rows
N = 8192               # features
BLOCK = 16
NB = N // BLOCK        # 512 blocks
EPS = 1e-5
R = B_FULL // N_CORES  # 512 rows per core

CW = 4096              # column chunk width


def build_nc(rows: int = R, cols: int = N, cw: int = CW) -> bass.Bass:
    nb = cols // BLOCK          # 512 blocks
    nrt = rows // 128           # 4 row tiles
    ncc = cols // cw            # chunks per row tile
    nbw = cw // BLOCK           # blocks per chunk
    spc = cw // 128             # 128-col sub-blocks per chunk
    mspc = 16                   # sub-blocks per mask accumulation group

    nc = bacc.Bacc("TRN2", target_bir_lowering=False, debug=False,
                   num_devices=N_CORES)
    x = nc.declare_dram_parameter("x", [rows, cols], BF16, isOutput=False)
    scn = nc.declare_dram_parameter("scn", [nb], F32, isOutput=False)
    shd = nc.declare_dram_parameter("shd", [2 * nb], BF16, isOutput=False)
    identbf = nc.declare_dram_parameter("identbf", [128, 128], BF16,
                                        isOutput=False)
    # maskall[f, k*128 + g] = 1 iff g == 8k + f//16 (16 sub-blocks per
    # 2048-col accumulation group)
    mask = nc.declare_dram_parameter("maskall", [128, mspc * 128], BF16,
                                     isOutput=False)
    out = nc.declare_dram_parameter("out", [rows, cols], BF16, isOutput=True)

    with tile.TileContext(nc) as tc:
        with (
            tc.tile_pool(name="singles", bufs=1) as singles,
            tc.tile_pool(name="xp", bufs=4) as xp,
            tc.tile_pool(name="sqp", bufs=2) as sqp,
            tc.tile_pool(name="trp", bufs=2) as trp,
            tc.tile_pool(name="mst", bufs=2) as mst,
            tc.tile_pool(name="cof", bufs=2) as cof,
            tc.tile_pool(name="psA", bufs=2, space="PSUM") as psA,
            tc.tile_pool(name="psB", bufs=2, space="PSUM") as psB,
        ):
            # constants: keep the sync queue free so x streams immediately
            # ident first: the very first transposes block on it, and the
            # scn/shd partition-broadcasts are slow 128-descriptor DMAs
            # that would otherwise delay it ~20us.
            ident_sb = singles.tile([128, 128], BF16)
            mask_sb = singles.tile([128, mspc * 128], BF16)
            nc.gpsimd.dma_start(out=ident_sb[:, :], in_=identbf[:, :])
            nc.gpsimd.dma_start(out=mask_sb[:, :], in_=mask[:, :])
            scn_sb = singles.tile([128, nb], F32)
            shd_sb = singles.tile([128, 2 * nb], BF16)
            nc.gpsimd.dma_start(out=scn_sb[:, :],
                                in_=scn[:].partition_broadcast(128))
            nc.gpsimd.dma_start(out=shd_sb[:, :],
                                in_=shd[:].partition_broadcast(128))
            eps_t = singles.tile([128, 1], F32)
            nc.vector.memset(eps_t[:, :], EPS)

            xts: dict = {}

            def stage_head(rt: int, c0: int, colw: int, s1_act: bool,
                           slice_dma: bool = False) -> dict:
                """DMA in + s1 stats + PE transposes + ACT squares."""
                r0 = rt * 128
                sl = slice(c0, c0 + colw)
                if c0 == 0:
                    xts[rt] = xp.tile([128, cols], BF16, tag="x",
                                      name=f"xt{rt}")
                xt = xts[rt]
                if slice_dma:
                    # slice the first chunk's DMA so compute starts early
                    for q in range(colw // 1024):
                        qs = slice(c0 + q * 1024, c0 + (q + 1) * 1024)
                        nc.sync.dma_start(out=xt[:, qs],
                                          in_=x[r0 : r0 + 128, qs])
                elif rt >= 2:
                    # throttle the back half's input DMAs through the
                    # scalar queue: dispatched only when ACT reaches this
                    # point in its queue, so the early wire isn't flooded
                    # with input traffic that stretches the first applies
                    nc.scalar.dma_start(out=xt[:, sl],
                                        in_=x[r0 : r0 + 128, sl])
                else:
                    nc.sync.dma_start(out=xt[:, sl],
                                      in_=x[r0 : r0 + 128, sl])
                # s1 path alternates per chunk to balance engines:
                #  - "tree": pairwise-halves adder tree on GpSimd
                #  - "act":  ACT copies the transposed raw data PSUM->SBUF
                #            and the Tensor engine mask-reduces it (PE
                #            issues a matmul every ~55ns, nearly free)
                nbw_c = colw // BLOCK
                uid = f"{rt}_{c0}"
                x3 = xt[:, sl].rearrange("p (g b) -> p g b", b=BLOCK)
                m_c = None
                xTs = None
                if not s1_act:
                    m_c = mst.tile([128, nbw_c], F32, tag="m",
                                   name=f"m_{uid}")
                    p1 = trp.tile([128, colw // 2], BF16, tag="p1",
                                  name=f"p1_{uid}")
                    p2 = trp.tile([128, colw // 4], BF16, tag="p2",
                                  name=f"p2_{uid}")
                    p3 = trp.tile([128, colw // 8], BF16, tag="p3",
                                  name=f"p3_{uid}")
                    nc.gpsimd.tensor_add(out=p1[:, :], in0=x3[:, :, 0:8],
                                         in1=x3[:, :, 8:16])
                    v1 = p1[:, :].rearrange("p (g b) -> p g b", b=8)
                    nc.gpsimd.tensor_add(out=p2[:, :], in0=v1[:, :, 0:4],
                                         in1=v1[:, :, 4:8])
                    v2 = p2[:, :].rearrange("p (g b) -> p g b", b=4)
                    nc.gpsimd.tensor_add(out=p3[:, :], in0=v2[:, :, 0:2],
                                         in1=v2[:, :, 2:4])
                    v3 = p3[:, :].rearrange("p (g b) -> p g b", b=2)
                    nc.gpsimd.tensor_add(out=m_c[:, :], in0=v3[:, :, 0:1],
                                         in1=v3[:, :, 1:2])
                else:
                    xTs = sqp.tile([128, colw], BF16, tag="xTs",
                                   name=f"xTs_{uid}")

                sqT = sqp.tile([128, colw], BF16, tag="sqT",
                               name=f"sqT_{uid}")
                for half in range(colw // 1024):
                    xT = psA.tile([128, 1024], BF16, tag="xT",
                                  name=f"xT_{uid}_{half}")
                    for j in range(8):
                        cj = c0 + half * 1024 + j * 128
                        nc.tensor.transpose(
                            xT[:, j * 128 : (j + 1) * 128],
                            xt[:, cj : cj + 128],
                            ident_sb[:, :],
                        )
                    nc.scalar.square(
                        out=sqT[:, half * 1024 : (half + 1) * 1024],
                        in_=xT[:, :],
                    )
                    if s1_act:
                        nc.scalar.copy(
                            out=xTs[:, half * 1024 : (half + 1) * 1024],
                            in_=xT[:, :],
                        )
                return {"rt": rt, "c0": c0, "colw": colw, "xt": xt,
                        "sqT": sqT, "xTs": xTs, "m_c": m_c}

            def stage_tail(st: dict, a_gps: bool = False) -> None:
                """PE masked matmuls + coeff + apply + DMA out (1 chunk
                behind stage_head so the engine streams stay separated)."""
                rt, c0, colw, xt, sqT, xTs, m_c = (
                    st["rt"], st["c0"], st["colw"], st["xt"],
                    st["sqT"], st["xTs"], st["m_c"])
                r0 = rt * 128
                sl = slice(c0, c0 + colw)
                nbw_c = colw // BLOCK
                spc_c = colw // 128
                uid = f"{rt}_{c0}"

                s2_ps = psB.tile([128, nbw_c], F32, tag="s2",
                                 name=f"s2_{uid}")
                gw = min(128, nbw_c)   # block-group (and moving) width
                for k in range(spc_c):
                    grp, mk = k // mspc, k % mspc
                    g0 = grp * 128
                    nc.tensor.matmul(
                        s2_ps[:, g0 : g0 + gw],
                        sqT[:, k * 128 : (k + 1) * 128],
                        mask_sb[:, mk * 128 : mk * 128 + gw],
                        start=(mk == 0),
                        stop=(mk == mspc - 1 or k == spc_c - 1),
                    )
                if xTs is not None:
                    s1_ps = psB.tile([128, nbw_c], F32, tag="s1",
                                     name=f"s1_{uid}")
                    for k in range(spc_c):
                        grp, mk = k // mspc, k % mspc
                        g0 = grp * 128
                        nc.tensor.matmul(
                            s1_ps[:, g0 : g0 + gw],
                            xTs[:, k * 128 : (k + 1) * 128],
                            mask_sb[:, mk * 128 : mk * 128 + gw],
                            start=(mk == 0),
                            stop=(mk == mspc - 1 or k == spc_c - 1),
                        )
                    s1_src = s1_ps
                else:
                    s1_src = m_c

                gbsl = slice(c0 // BLOCK, c0 // BLOCK + nbw_c)
                mm = cof.tile([128, nbw_c], F32, tag="mm", name=f"mm_{uid}")
                raw = cof.tile([128, nbw_c], F32, tag="raw",
                               name=f"raw_{uid}")
                sd = cof.tile([128, nbw_c], F32, tag="sd", name=f"sd_{uid}")
                rstd = cof.tile([128, nbw_c], F32, tag="rstd",
                                name=f"rstd_{uid}")
                t_dup = cof.tile([128, 2 * nbw_c], BF16, tag="td",
                                 name=f"td_{uid}")
                amr_acc = cof.tile([128, 1], F32, tag="acc",
                                   name=f"acc_{uid}")
                a_dup = cof.tile([128, 2 * nbw_c], BF16, tag="ad",
                                 name=f"ad_{uid}")
                b_dup = cof.tile([128, 2 * nbw_c], BF16, tag="bd",
                                 name=f"bd_{uid}")

                nc.scalar.square(out=mm[:, :], in_=s1_src[:, :])
                nc.vector.scalar_tensor_tensor(
                    out=raw[:, :], in0=mm[:, :], scalar=-1.0 / BLOCK,
                    in1=s2_ps[:, :], op0=ALU.mult, op1=ALU.add,
                )
                nc.scalar.activation(
                    out=sd[:, :], in_=raw[:, :],
                    func=mybir.ActivationFunctionType.Sqrt,
                    bias=eps_t[:, :], scale=1.0 / (BLOCK - 1),
                )
                nc.vector.reciprocal_approx_fast(out=rstd[:, :],
                                                 in_=sd[:, :])
                a_eng = nc.gpsimd if a_gps else nc.vector
                a_eng.tensor_mul(
                    out=a_dup[:, :].rearrange("p (g e) -> p g e", e=2),
                    in0=scn_sb[:, gbsl].unsqueeze(2)
                        .broadcast_to((128, nbw_c, 2)),
                    in1=rstd[:, :].unsqueeze(2)
                        .broadcast_to((128, nbw_c, 2)),
                )
                # t_dup = (s1 * -1/16) * a  (dup'd, bf16) in one custom op,
                # then b = shifts + t_dup as a packed-bf16 2x add
                nc.vector.affine_mul_reduce(
                    out=t_dup[:, :].rearrange("p (g e) -> p g e", e=2),
                    accum_out=amr_acc[:, :],
                    in0=s1_src[:, :].unsqueeze(2)
                        .broadcast_to((128, nbw_c, 2)),
                    in1=a_dup[:, :].rearrange("p (g e) -> p g e", e=2),
                    scale=-1.0 / BLOCK, bias=0.0,
                )
                nc.vector.tensor_add(
                    out=b_dup[:, :], in0=t_dup[:, :],
                    in1=shd_sb[:, c0 // 8 : c0 // 8 + 2 * nbw_c],
                )

                x4 = xt[:, sl].rearrange("p (g b8 e) -> p g b8 e",
                                         b8=8, e=2)
                a4 = (a_dup[:, :].rearrange("p (g e) -> p g e", e=2)
                      .unsqueeze(2).broadcast_to((128, nbw_c, 8, 2)))
                b4 = (b_dup[:, :].rearrange("p (g e) -> p g e", e=2)
                      .unsqueeze(2).broadcast_to((128, nbw_c, 8, 2)))
                ap_eng = nc.gpsimd if st.get("apply_gps") else nc.vector
                ap_eng.tensor_mul(out=x4, in0=x4, in1=a4)
                ap_eng.tensor_add(out=x4, in0=x4, in1=b4)
                nc.sync.dma_start(out=out[r0 : r0 + 128, sl],
                                  in_=xt[:, sl])

            # chunk schedule: the first row-tile runs at 2048-col width so
            # the pipeline ramps quickly; the rest at 4096.  A 1-chunk lag
            # between head and tail keeps the head-stage streams
            # (DMA/tree/transpose/square) time-shifted from the tail-stage
            # streams (matmul-read/coeff/apply) — concurrent same-tile
            # streams contend for SBUF bandwidth and stretch everything.
            # Chunk 0's tail is emitted immediately so the DVE doesn't
            # idle through the fill; the last two chunks' a_dup runs on
            # the (by then idle) GpSimd.
            chunks = []
            for rt in range(nrt):
                cc0 = 0
                for _ in range(ncc):
                    chunks.append((rt, cc0, cw))
                    cc0 += cw
            prev = None
            for i, (rt, cc0, w) in enumerate(chunks):
                s1_act = (i % 2) == 0
                st = stage_head(rt, cc0, w, s1_act, slice_dma=(i == 0))
                if i == 0:
                    stage_tail(st)
                else:
                    if prev is not None:
                        stage_tail(prev)
                    prev = st
            if prev is not None:
                stage_tail(prev)
    nc.compile()
    return nc


def aux_inputs() -> dict:
    """Constant tensors fed alongside the real inputs."""
    mspc = 16
    maskall = np.zeros((128, mspc * 128), np.float32)
    for k in range(mspc):
        for f in range(128):
            maskall[f, k * 128 + 8 * k + f // BLOCK] = 1.0
    return {
        "identbf": np.eye(128, dtype=np.float32).astype(ml_dtypes.bfloat16),
        "maskall": maskall.astype(ml_dtypes.bfloat16),
    }


_NC_CACHE: dict = {}


def _get_nc() -> bass.Bass:
    if "nc" not in _NC_CACHE:
        _NC_CACHE["nc"] = build_nc()
    return _NC_CACHE["nc"]


def run_sharded(x, scales, shifts, trace: bool = False):
    """Run the SPMD kernel on 8 cores. Returns (out, BassKernelResults)."""
    x = np.ascontiguousarray(np.asarray(x, dtype=np.float32))
    scales = np.ascontiguousarray(np.asarray(scales, dtype=np.float32))
    shifts = np.ascontiguousarray(np.asarray(shifts, dtype=np.float32))
    assert x.shape == (B_FULL, N), x.shape
    xb = x.astype(ml_dtypes.bfloat16)
    shd = np.repeat(shifts, 2).astype(ml_dtypes.bfloat16)
    nc = _get_nc()
    aux = aux_inputs()
    in_maps = [
        {"x": xb[i * R : (i + 1) * R], "scn": scales, "shd": shd, **aux}
        for i in range(N_CORES)
    ]
    res = run_bass_kernel_spmd(nc, in_maps, core_ids=list(range(N_CORES)),
                               trace=trace)
    outs = [np.asarray(m["out"]).astype(np.float32) for m in res.results]
    return np.concatenate(outs, axis=0), res


def kernel(x, scales, shifts):
    out, _ = run_sharded(x, scales, shifts, trace=False)
    return out


# revision 63
# speedup vs baseline: 1.0240x; 1.0240x over previous
"""Blockwise reconditioner (block-16 normalization) on 8 Trainium2 cores.

Math per row r, block g (block size 16):
    mean = mean(x[r, 16g:16g+16])
    var  = sum((x - mean)^2) / 15          (unbiased, ddof=1)
    out  = (x - mean) / sqrt(var + 1e-5) * scales[g] + shifts[g]

Implemented as out = x * a + b with per-block coefficients
    a = scales[g] / sqrt(var + eps)
    b = shifts[g] - mean * a
using raw = sum(x^2) - s1^2/16, var = raw/15 (s1 = block sum).

bf16 data path (tolerance is 2e-2; bf16 end-to-end measures ~3.4e-3):
host casts x fp32 -> bf16, device computes in bf16/fp32, host casts the
bf16 output back to fp32.  Halves HBM traffic (DMA roofline ~94us fp32
-> ~47us bf16) and enables the DVE 2x perf mode for the apply.

Per-core pipeline over 8 flattened [128, 4096] chunks, software
pipelined with a 1-chunk lag between a chunk's "head" (DMA in, s1
stats, PE transposes, ACT squares) and its "tail" (PE masked matmuls,
coefficient math, apply, DMA out).  The lag keeps the head-stage and
tail-stage SBUF streams time-shifted — concurrent same-tile streams
contend for SBUF bandwidth and stretch every engine's ops ~1.5x — and
keeps each engine's in-order queue free of cross-engine stalls.

s1 (block sums) alternate per chunk between two engines to balance
load: GpSimd (4-pass pairwise-halves TENSOR_TENSOR adder tree;
InstPool/STT are invalid on the Pool engine, and TensorReduce gets no
bf16 speedup on the DVE) and ACT (copy the PE-transposed raw data
PSUM->SBUF; the Tensor engine then mask-reduces it — it issues a
matmul every ~55ns fully pipelined, so extra matmul passes are nearly
free).  s2 always: ACT squares PSUM->SBUF bf16, then 32 matmuls
(stationary=sqT_k, moving=mask_k) accumulate row-major [row, block]
sums directly in PSUM (no flip back needed).

Coefficients per chunk: raw = s2 - s1^2/16 (STT against PSUM),
sd = sqrt(raw/15 + eps) (ACT), rstd via the custom-DVE fast
reciprocal, a/b written DUPLICATED (a[2g]=a[2g+1]) in bf16 so the
apply's broadcast AP keeps a packed 2-byte last dim; b is fused via
the AFFINE_MUL_REDUCE custom op + one packed-bf16 add.  The apply
out = x*a + b runs in place as two DVE TENSOR_TENSOR passes in 2x_1p
mode, then DMA out.

Measured: 161.3us fp32 baseline -> 109.6us (DVE is the end-to-end
governor: ~26us to first coefficient, then ~78us dense DVE work).
"""

import sys

import numpy as np
import ml_dtypes

for _p in ("/opt/trn_rl_repo",):
    if _p not in sys.path:
        sys.path.insert(0, _p)

import concourse.bacc as bacc
import concourse.bass as bass
import concourse.tile as tile
from concourse import mybir
from concourse.bass_utils import run_bass_kernel_spmd

F32 = mybir.dt.float32
BF16 = mybir.dt.bfloat16
ALU = mybir.AluOpType

N_CORES = 8
B_FULL = 4096          # total rows
N = 8192               # features
BLOCK = 16
NB = N // BLOCK        # 512 blocks
EPS = 1e-5
R = B_FULL // N_CORES  # 512 rows per core

CW = 4096              # column chunk width


def build_nc(rows: int = R, cols: int = N, cw: int = CW) -> bass.Bass:
    nb = cols // BLOCK          # 512 blocks
    nrt = rows // 128           # 4 row tiles
    ncc = cols // cw            # chunks per row tile
    nbw = cw // BLOCK           # blocks per chunk
    spc = cw // 128             # 128-col sub-blocks per chunk
    mspc = 16                   # sub-blocks per mask accumulation group

    nc = bacc.Bacc("TRN2", target_bir_lowering=False, debug=False,
                   num_devices=N_CORES)
    x = nc.declare_dram_parameter("x", [rows, cols], BF16, isOutput=False)
    scn = nc.declare_dram_parameter("scn", [nb], F32, isOutput=False)
    shd = nc.declare_dram_parameter("shd", [2 * nb], BF16, isOutput=False)
    identbf = nc.declare_dram_parameter("identbf", [128, 128], BF16,
                                        isOutput=False)
    # maskall[f, k*128 + g] = 1 iff g == 8k + f//16 (16 sub-blocks per
    # 2048-col accumulation group)
    mask = nc.declare_dram_parameter("maskall", [128, mspc * 128], BF16,
                                     isOutput=False)
    out = nc.declare_dram_parameter("out", [rows, cols], BF16, isOutput=True)

    with tile.TileContext(nc) as tc:
        with (
            tc.tile_pool(name="singles", bufs=1) as singles,
            tc.tile_pool(name="xp", bufs=4) as xp,
            tc.tile_pool(name="sqp", bufs=2) as sqp,
            tc.tile_pool(name="trp", bufs=2) as trp,
            tc.tile_pool(name="mst", bufs=2) as mst,
            tc.tile_pool(name="cof", bufs=2) as cof,
            tc.tile_pool(name="psA", bufs=2, space="PSUM") as psA,
            tc.tile_pool(name="psB", bufs=2, space="PSUM") as psB,
        ):
            # constants: keep the sync queue free so x streams immediately
            # ident first: the very first transposes block on it, and the
            # scn/shd partition-broadcasts are slow 128-descriptor DMAs
            # that would otherwise delay it ~20us.
            ident_sb = singles.tile([128, 128], BF16)
            mask_sb = singles.tile([128, mspc * 128], BF16)
            nc.gpsimd.dma_start(out=ident_sb[:, :], in_=identbf[:, :])
            nc.gpsimd.dma_start(out=mask_sb[:, :], in_=mask[:, :])
            scn_sb = singles.tile([128, nb], F32)
            shd_sb = singles.tile([128, 2 * nb], BF16)
            nc.gpsimd.dma_start(out=scn_sb[:, :],
                                in_=scn[:].partition_broadcast(128))
            nc.gpsimd.dma_start(out=shd_sb[:, :],
                                in_=shd[:].partition_broadcast(128))
            eps_t = singles.tile([128, 1], F32)
            nc.vector.memset(eps_t[:, :], EPS)

            xts: dict = {}

            def stage_head(rt: int, c0: int, colw: int, s1_act: bool,
                           slice_dma: bool = False) -> dict:
                """DMA in + s1 stats + PE transposes + ACT squares."""
                r0 = rt * 128
                sl = slice(c0, c0 + colw)
                if c0 == 0:
                    xts[rt] = xp.tile([128, cols], BF16, tag="x",
                                      name=f"xt{rt}")
                xt = xts[rt]
                if slice_dma:
                    # slice the first chunk's DMA so compute starts early
                    for q in range(colw // 1024):
                        qs = slice(c0 + q * 1024, c0 + (q + 1) * 1024)
                        nc.sync.dma_start(out=xt[:, qs],
                                          in_=x[r0 : r0 + 128, qs])
                else:
                    nc.sync.dma_start(out=xt[:, sl],
                                      in_=x[r0 : r0 + 128, sl])
                # s1 path alternates per chunk to balance engines:
                #  - "tree": pairwise-halves adder tree on GpSimd
                #  - "act":  ACT copies the transposed raw data PSUM->SBUF
                #            and the Tensor engine mask-reduces it (PE
                #            issues a matmul every ~55ns, nearly free)
                nbw_c = colw // BLOCK
                uid = f"{rt}_{c0}"
                x3 = xt[:, sl].rearrange("p (g b) -> p g b", b=BLOCK)
                m_c = None
                xTs = None
                if not s1_act:
                    m_c = mst.tile([128, nbw_c], F32, tag="m",
                                   name=f"m_{uid}")
                    p1 = trp.tile([128, colw // 2], BF16, tag="p1",
                                  name=f"p1_{uid}")
                    p2 = trp.tile([128, colw // 4], BF16, tag="p2",
                                  name=f"p2_{uid}")
                    p3 = trp.tile([128, colw // 8], BF16, tag="p3",
                                  name=f"p3_{uid}")
                    nc.gpsimd.tensor_add(out=p1[:, :], in0=x3[:, :, 0:8],
                                         in1=x3[:, :, 8:16])
                    v1 = p1[:, :].rearrange("p (g b) -> p g b", b=8)
                    nc.gpsimd.tensor_add(out=p2[:, :], in0=v1[:, :, 0:4],
                                         in1=v1[:, :, 4:8])
                    v2 = p2[:, :].rearrange("p (g b) -> p g b", b=4)
                    nc.gpsimd.tensor_add(out=p3[:, :], in0=v2[:, :, 0:2],
                                         in1=v2[:, :, 2:4])
                    v3 = p3[:, :].rearrange("p (g b) -> p g b", b=2)
                    nc.gpsimd.tensor_add(out=m_c[:, :], in0=v3[:, :, 0:1],
                                         in1=v3[:, :, 1:2])
                else:
                    xTs = sqp.tile([128, colw], BF16, tag="xTs",
                                   name=f"xTs_{uid}")

                sqT = sqp.tile([128, colw], BF16, tag="sqT",
                               name=f"sqT_{uid}")
                for half in range(colw // 1024):
                    xT = psA.tile([128, 1024], BF16, tag="xT",
                                  name=f"xT_{uid}_{half}")
                    for j in range(8):
                        cj = c0 + half * 1024 + j * 128
                        nc.tensor.transpose(
                            xT[:, j * 128 : (j + 1) * 128],
                            xt[:, cj : cj + 128],
                            ident_sb[:, :],
                        )
                    nc.scalar.square(
                        out=sqT[:, half * 1024 : (half + 1) * 1024],
                        in_=xT[:, :],
                    )
                    if s1_act:
                        nc.scalar.copy(
                            out=xTs[:, half * 1024 : (half + 1) * 1024],
                            in_=xT[:, :],
                        )
                return {"rt": rt, "c0": c0, "colw": colw, "xt": xt,
                        "sqT": sqT, "xTs": xTs, "m_c": m_c}

            def stage_tail(st: dict, a_gps: bool = False) -> None:
                """PE masked matmuls + coeff + apply + DMA out (1 chunk
                behind stage_head so the engine streams stay separated)."""
                rt, c0, colw, xt, sqT, xTs, m_c = (
                    st["rt"], st["c0"], st["colw"], st["xt"],
                    st["sqT"], st["xTs"], st["m_c"])
                r0 = rt * 128
                sl = slice(c0, c0 + colw)
                nbw_c = colw // BLOCK
                spc_c = colw // 128
                uid = f"{rt}_{c0}"

                s2_ps = psB.tile([128, nbw_c], F32, tag="s2",
                                 name=f"s2_{uid}")
                gw = min(128, nbw_c)   # block-group (and moving) width
                for k in range(spc_c):
                    grp, mk = k // mspc, k % mspc
                    g0 = grp * 128
                    nc.tensor.matmul(
                        s2_ps[:, g0 : g0 + gw],
                        sqT[:, k * 128 : (k + 1) * 128],
                        mask_sb[:, mk * 128 : mk * 128 + gw],
                        start=(mk == 0),
                        stop=(mk == mspc - 1 or k == spc_c - 1),
                    )
                if xTs is not None:
                    s1_ps = psB.tile([128, nbw_c], F32, tag="s1",
                                     name=f"s1_{uid}")
                    for k in range(spc_c):
                        grp, mk = k // mspc, k % mspc
                        g0 = grp * 128
                        nc.tensor.matmul(
                            s1_ps[:, g0 : g0 + gw],
                            xTs[:, k * 128 : (k + 1) * 128],
                            mask_sb[:, mk * 128 : mk * 128 + gw],
                            start=(mk == 0),
                            stop=(mk == mspc - 1 or k == spc_c - 1),
                        )
                    s1_src = s1_ps
                else:
                    s1_src = m_c

                gbsl = slice(c0 // BLOCK, c0 // BLOCK + nbw_c)
                mm = cof.tile([128, nbw_c], F32, tag="mm", name=f"mm_{uid}")
                raw = cof.tile([128, nbw_c], F32, tag="raw",
                               name=f"raw_{uid}")
                sd = cof.tile([128, nbw_c], F32, tag="sd", name=f"sd_{uid}")
                rstd = cof.tile([128, nbw_c], F32, tag="rstd",
                                name=f"rstd_{uid}")
                t_dup = cof.tile([128, 2 * nbw_c], BF16, tag="td",
                                 name=f"td_{uid}")
                amr_acc = cof.tile([128, 1], F32, tag="acc",
                                   name=f"acc_{uid}")
                a_dup = cof.tile([128, 2 * nbw_c], BF16, tag="ad",
                                 name=f"ad_{uid}")
                b_dup = cof.tile([128, 2 * nbw_c], BF16, tag="bd",
                                 name=f"bd_{uid}")

                nc.scalar.square(out=mm[:, :], in_=s1_src[:, :])
                nc.vector.scalar_tensor_tensor(
                    out=raw[:, :], in0=mm[:, :], scalar=-1.0 / BLOCK,
                    in1=s2_ps[:, :], op0=ALU.mult, op1=ALU.add,
                )
                nc.scalar.activation(
                    out=sd[:, :], in_=raw[:, :],
                    func=mybir.ActivationFunctionType.Sqrt,
                    bias=eps_t[:, :], scale=1.0 / (BLOCK - 1),
                )
                nc.vector.reciprocal_approx_fast(out=rstd[:, :],
                                                 in_=sd[:, :])
                a_eng = nc.gpsimd if a_gps else nc.vector
                a_eng.tensor_mul(
                    out=a_dup[:, :].rearrange("p (g e) -> p g e", e=2),
                    in0=scn_sb[:, gbsl].unsqueeze(2)
                        .broadcast_to((128, nbw_c, 2)),
                    in1=rstd[:, :].unsqueeze(2)
                        .broadcast_to((128, nbw_c, 2)),
                )
                # t_dup = (s1 * -1/16) * a  (dup'd, bf16) in one custom op,
                # then b = shifts + t_dup as a packed-bf16 2x add
                nc.vector.affine_mul_reduce(
                    out=t_dup[:, :].rearrange("p (g e) -> p g e", e=2),
                    accum_out=amr_acc[:, :],
                    in0=s1_src[:, :].unsqueeze(2)
                        .broadcast_to((128, nbw_c, 2)),
                    in1=a_dup[:, :].rearrange("p (g e) -> p g e", e=2),
                    scale=-1.0 / BLOCK, bias=0.0,
                )
                nc.vector.tensor_add(
                    out=b_dup[:, :], in0=t_dup[:, :],
                    in1=shd_sb[:, c0 // 8 : c0 // 8 + 2 * nbw_c],
                )

                x4 = xt[:, sl].rearrange("p (g b8 e) -> p g b8 e",
                                         b8=8, e=2)
                a4 = (a_dup[:, :].rearrange("p (g e) -> p g e", e=2)
                      .unsqueeze(2).broadcast_to((128, nbw_c, 8, 2)))
                b4 = (b_dup[:, :].rearrange("p (g e) -> p g e", e=2)
                      .unsqueeze(2).broadcast_to((128, nbw_c, 8, 2)))
                ap_eng = nc.gpsimd if st.get("apply_gps") else nc.vector
                ap_eng.tensor_mul(out=x4, in0=x4, in1=a4)
                ap_eng.tensor_add(out=x4, in0=x4, in1=b4)
                nc.sync.dma_start(out=out[r0 : r0 + 128, sl],
                                  in_=xt[:, sl])

            # chunk schedule: the first row-tile runs at 2048-col width so
            # the pipeline ramps quickly; the rest at 4096.  A 1-chunk lag
            # between head and tail keeps the head-stage streams
            # (DMA/tree/transpose/square) time-shifted from the tail-stage
            # streams (matmul-read/coeff/apply) — concurrent same-tile
            # streams contend for SBUF bandwidth and stretch everything.
            # Chunk 0's tail is emitted immediately so the DVE doesn't
            # idle through the fill; the last two chunks' a_dup runs on
            # the (by then idle) GpSimd.
            chunks = []
            for rt in range(nrt):
                cc0 = 0
                for _ in range(ncc):
                    chunks.append((rt, cc0, cw))
                    cc0 += cw
            prev = None
            for i, (rt, cc0, w) in enumerate(chunks):
                s1_act = (i % 2) == 0
                st = stage_head(rt, cc0, w, s1_act, slice_dma=(i == 0))
                if i == 0:
                    stage_tail(st)
                else:
                    if prev is not None:
                        stage_tail(prev)
                    prev = st
            if prev is not None:
                stage_tail(prev)
    nc.compile()
    return nc


def aux_inputs() -> dict:
    """Constant tensors fed alongside the real inputs."""
    mspc = 16
    maskall = np.zeros((128, mspc * 128), np.float32)
    for k in range(mspc):
        for f in range(128):
            maskall[f, k * 128 + 8 * k + f // BLOCK] = 1.0
    return {
        "identbf": np.eye(128, dtype=np.float32).astype(ml_dtypes.bfloat16),
        "maskall": maskall.astype(ml_dtypes.bfloat16),
    }


_NC_CACHE: dict = {}


def _get_nc() -> bass.Bass:
    if "nc" not in _NC_CACHE:
        _NC_CACHE["nc"] = build_nc()
    return _NC_CACHE["nc"]


def run_sharded(x, scales, shifts, trace: bool = False):
    """Run the SPMD kernel on 8 cores. Returns (out, BassKernelResults)."""
    x = np.ascontiguousarray(np.asarray(x, dtype=np.float32))
    scales = np.ascontiguousarray(np.asarray(scales, dtype=np.float32))
    shifts = np.ascontiguousarray(np.asarray(shifts, dtype=np.float32))
    assert x.shape == (B_FULL, N), x.shape
    xb = x.astype(ml_dtypes.bfloat16)
    shd = np.repeat(shifts, 2).astype(ml_dtypes.bfloat16)
    nc = _get_nc()
    aux = aux_inputs()
    in_maps = [
        {"x": xb[i * R : (i + 1) * R], "scn": scales, "shd": shd, **aux}
        for i in range(N_CORES)
    ]
    res = run_bass_kernel_spmd(nc, in_maps, core_ids=list(range(N_CORES)),
                               trace=trace)
    outs = [np.asarray(m["out"]).astype(np.float32) for m in res.results]
    return np.concatenate(outs, axis=0), res


def kernel(x, scales, shifts):
    out, _ = run_sharded(x, scales, shifts, trace=False)
    return out
